# revision 1
# baseline (speedup 1.0000x reference)
"""Trainium2 Bass kernel for nn_Conv_6511170421767.

3x3 conv, stride 1, pad 1 on x:(32,128,56,56) with weight:(256,128,3,3),
bias:(256,) -> out:(32,256,56,56), fp32 in/out.

Strategy (data-parallel, 4 images per core on 8 cores):
- Cin=128 is exactly the PE contraction/partition dim. The conv becomes,
  per (output-row-block, Cout-chunk), an accumulation of 9 matmuls (one per
  kernel tap): out[co, pix] += W[dr,dc][ci,co].T @ xpad[ci, shifted pix].
- x is zero-padded once into SBUF as [128, 58, 58] per image; a matmul rhs
  slice [128, (8 rows x 58 stride), 56] walks the padded plane, so no edge
  fix-ups are needed. Only the 1-wide borders are zeroed (once); the
  interior is overwritten per image.
- Matmul operands are DVE-rounded to fp16 (1 PE cycle/row like bf16 - vs 4
  for plain fp32 - but with a 10-bit mantissa; operand ranges here sit
  safely inside fp16's dynamic range). Accumulation is fp32 in PSUM.
  Measured vs the fp32 reference: rel err 2.9e-4 (bf16: 2.2e-3, float32r:
  1.4e-4 but ~14us slower from its per-matmul weight-reload shadow).
- PSUM tile [128, 448] = one bank; 9 taps accumulate in-bank, then the
  scalar engine adds bias (Identity activation w/ per-partition bias AP)
  while copying PSUM->SBUF, and the result DMAs out on the sync queue.
- Measured on 8 axon-tunneled trn2 cores: ~118us HW exec per core
  (PE matmul busy ~99us = the N/2.4GHz streaming wall for 504 matmuls of
  N=448; plus ~7.5us fixed framework preamble, ~3us ramp, ~3.5us tail).

The external neuronxcc walrus in this container enforces small per-
instruction sync-wait limits (Matmult/S3_LW fails at 2 waits,
TensorCopy/S4D4_TR at 2, Drain/CTRL_NO at 5 - TRN2 HW allows 1 per
bacc.generate_event_semaphores). Tile emits up to ~10 waits on the final
drain, so _cap_sync_waits() splits excess waits onto InstNoOp instructions
inserted just before the offender on the same engine.
"""

import sys

sys.path.insert(0, "/opt/trn_rl_repo")

import numpy as np

import concourse.bass as bass
import concourse.mybir as mybir
import concourse.tile as tile
from concourse.bass_utils import run_bass_kernel_spmd

F32 = mybir.dt.float32
F32R = mybir.dt.float32r
BF16 = mybir.dt.bfloat16
FP16 = mybir.dt.float16

# "fp16": fp16 matmul, 1 PE cycle/row, rel err ~2.9e-4  <- shipped
# "f32r": full-rate fp32 matmul (rel err ~1.4e-4, ~214ns/MM, ~+9us)
# "bf16": bf16 matmul (rel err ~2.2e-3, same speed as fp16)
VARIANT = "fp16"

N_CORES = 8
IMGS_PER_CORE = 4
CIN = 128
COUT = 256
H = W = 56
HP = WP = 58  # padded plane
ROWS_PER_TILE = 8  # 8 output rows -> N = 448 <= 512 (one PSUM bank)
N_ROW_TILES = H // ROWS_PER_TILE  # 7
NTILE = ROWS_PER_TILE * W  # 448

# Per-instruction sync-wait budget for the external walrus: TRN2 hardware
# allows at most 1 sync wait per instruction (bacc.generate_event_semaphores
# doc); observed failures: Matmult/S3_LW at 2, TensorCopy/S4D4_TR at 2,
# Drain/CTRL_NO at 5.
_WAIT_LIMITS_DEFAULT = 1
_WAIT_LIMITS = {}


def _cap_sync_waits(nc):
    """Split sync waits exceeding per-instruction limits onto same-engine
    InstNoOp instructions inserted immediately before the offender."""
    for fn in nc.m.functions:
        for bb in fn.blocks:
            i = 0
            insts = bb.instructions
            while i < len(insts):
                inst = insts[i]
                si = getattr(inst, "sync_info", None)
                if si is None or not si.on_wait:
                    i += 1
                    continue
                limit = _WAIT_LIMITS.get(type(inst).__name__, _WAIT_LIMITS_DEFAULT)
                waits = list(si.on_wait)
                if len(waits) <= limit:
                    i += 1
                    continue
                keep = waits[:limit]
                excess = waits[limit:]
                inst.sync_info = mybir.SyncInfo(
                    on_wait=keep, on_update=list(si.on_update)
                )
                pos = i
                for j in range(0, len(excess), _WAIT_LIMITS_DEFAULT):
                    chunk = excess[j : j + _WAIT_LIMITS_DEFAULT]
                    nop = mybir.InstNoOp(
                        name=nc.get_next_instruction_name(), ins=[], outs=[]
                    )
                    nop.engine = inst.engine
                    nop.sync_info = mybir.SyncInfo(on_wait=chunk, on_update=[])
                    nc.register_instruction(nop)
                    insts.insert(pos, nop)
                    pos += 1
                    i += 1
                i += 1


def build_conv_nc():
    """One-core program: x:(4,128,56,56) w/ wT:(128,9,256), bias2:(128,2)
    -> out:(4,256,56,56)."""
    nc = bass.Bass()
    MMDT = {"f32r": F32R, "bf16": BF16, "fp16": FP16}[VARIANT]
    x = nc.dram_tensor("x", [IMGS_PER_CORE, CIN, H, W], F32, kind="ExternalInput")
    wt = nc.dram_tensor("wT", [CIN, 9, COUT], F32, kind="ExternalInput")
    bias2 = nc.dram_tensor("bias2", [128, 2], F32, kind="ExternalInput")
    out = nc.dram_tensor(
        "out", [IMGS_PER_CORE, COUT, H, W], F32, kind="ExternalOutput"
    )

    with tile.TileContext(nc) as tc:
        with (
            tc.tile_pool(name="const", bufs=1) as const_pool,
            tc.tile_pool(name="xpad", bufs=1) as xpad_pool,
            tc.tile_pool(name="xstage", bufs=4) as xstage_pool,
            tc.tile_pool(name="obuf", bufs=4) as obuf_pool,
            tc.tile_pool(name="psum", bufs=8, space="PSUM") as psum_pool,
        ):
            # Weights: HWDGE DMA per tap into an f32 stage, DVE-round into
            # the matmul dtype. Per-tap split lets the first matmul start
            # ~1us after the preamble.
            wt3 = wt  # [CIN, 9, COUT]
            w_stage = const_pool.tile([CIN, 9, COUT], F32)
            w_sb = const_pool.tile([CIN, 9 * COUT], MMDT)
            zt = const_pool.tile([CIN, HP], F32)
            xpads = [
                xpad_pool.tile([CIN, HP, WP], MMDT, tag=f"xpad{bi}", name=f"xpad{bi}")
                for bi in range(2)
            ]

            def w_tap(k):
                nc.sync.dma_start(w_stage[:, k, :], wt3[:, k, :])
                nc.vector.tensor_copy(
                    w_sb[:, k * COUT : (k + 1) * COUT], w_stage[:, k, :]
                )

            def zero_borders(xp):
                # Only the 1-wide borders need zeroing (interior is fully
                # overwritten per image). memset can't write f32r, so zero
                # a small f32 tile and DVE-copy (which rounds) into the
                # four border strips.
                nc.vector.tensor_copy(xp[:, 0, :], zt[:])          # top row
                nc.vector.tensor_copy(xp[:, HP - 1, :], zt[:])     # bottom
                nc.vector.tensor_copy(xp[:, 1 : HP - 1, 0], zt[:, : HP - 2])
                nc.vector.tensor_copy(xp[:, 1 : HP - 1, WP - 1], zt[:, : HP - 2])

            def x_tile(img, t):
                # Scalar-engine HWDGE DMA per row-tile into an f32 stage,
                # then DVE-round into the padded interior. Scalar's queue
                # runs parallel to sync's w/out queue.
                xp = xpads[img % 2]
                y0 = t * ROWS_PER_TILE
                xs = xstage_pool.tile(
                    [CIN, ROWS_PER_TILE, W], F32, tag="xs", name=f"xs_{img}_{t}"
                )
                nc.scalar.dma_start(xs[:], x[img, :, y0 : y0 + ROWS_PER_TILE, :])
                nc.vector.tensor_copy(
                    xp[:, y0 + 1 : y0 + 1 + ROWS_PER_TILE, 1 : W + 1], xs[:]
                )

            # Startup, ordered for the PE ramp: the DVE instruction stream
            # is static, so interleave w-tap casts with the first image's
            # row-tile casts in consumption order.
            nc.vector.memset(zt[:], 0.0)
            w_tap(0)
            zero_borders(xpads[0])
            x_tile(0, 0)
            w_tap(1)
            w_tap(2)
            x_tile(0, 1)
            w_tap(3)
            w_tap(4)
            x_tile(0, 2)
            w_tap(5)
            w_tap(6)
            x_tile(0, 3)
            w_tap(7)
            w_tap(8)
            b_sb = const_pool.tile([128, 2], F32)
            nc.sync.dma_start(b_sb[:], bias2[:])
            for t in range(4, N_ROW_TILES):
                x_tile(0, t)
            zero_borders(xpads[1])

            for img in range(IMGS_PER_CORE):
                xp = xpads[img % 2]
                if img > 0:
                    for t in range(N_ROW_TILES):
                        x_tile(img, t)

                for t in range(N_ROW_TILES):
                    y0 = t * ROWS_PER_TILE
                    for c in range(2):  # Cout chunks of 128
                        ps = psum_pool.tile(
                            [128, NTILE], F32, tag="ps", name=f"ps_{img}_{t}_{c}"
                        )
                        k = 0
                        for dr in range(3):
                            for dc in range(3):
                                lhsT = w_sb[
                                    :,
                                    (dr * 3 + dc) * COUT
                                    + c * 128 : (dr * 3 + dc) * COUT
                                    + c * 128
                                    + 128,
                                ]
                                rhs = xp[
                                    :,
                                    y0 + dr : y0 + dr + ROWS_PER_TILE,
                                    dc : dc + W,
                                ]
                                nc.tensor.matmul(
                                    ps[:],
                                    lhsT,
                                    rhs,
                                    start=(k == 0),
                                    stop=(k == 8),
                                )
                                k += 1
                        ob = obuf_pool.tile(
                            [128, ROWS_PER_TILE, W], F32, tag="ob",
                            name=f"ob_{img}_{t}_{c}",
                        )
                        # out = Identity(psum * 1.0 + bias[co]) on ScalarE
                        nc.scalar.activation(
                            ob[:],
                            ps[:].rearrange("p (r w) -> p r w", w=W),
                            mybir.ActivationFunctionType.Identity,
                            bias=b_sb[:, c : c + 1],
                            scale=1.0,
                        )
                        nc.sync.dma_start(
                            out[img, c * 128 : (c + 1) * 128, y0 : y0 + ROWS_PER_TILE, :],
                            ob[:],
                        )

    _cap_sync_waits(nc)
    nc.finalize()
    return nc


_NC_CACHE = {}


def _get_nc():
    if "nc" not in _NC_CACHE:
        _NC_CACHE["nc"] = build_conv_nc()
    return _NC_CACHE["nc"]


def _prep_in_maps(x, weight, bias):
    x = np.ascontiguousarray(x, dtype=np.float32)
    # weight (256,128,3,3) -> wT[ci, dr*3+dc, co]
    wT = np.ascontiguousarray(
        np.transpose(np.asarray(weight, dtype=np.float32), (1, 2, 3, 0)).reshape(
            CIN, 9, COUT
        )
    )
    bias2 = np.ascontiguousarray(
        np.asarray(bias, dtype=np.float32).reshape(2, 128).T
    )
    per_core = x.shape[0] // N_CORES
    return [
        {
            "x": x[i * per_core : (i + 1) * per_core],
            "wT": wT,
            "bias2": bias2,
        }
        for i in range(N_CORES)
    ]


def run(x, weight, bias, trace=False):
    """Run the conv on 8 cores; returns (out, BassKernelResults)."""
    nc = _get_nc()
    in_maps = _prep_in_maps(x, weight, bias)
    res = run_bass_kernel_spmd(
        nc, in_maps, core_ids=list(range(N_CORES)), trace=trace
    )
    out = np.concatenate([r["out"] for r in res.results], axis=0)
    return out, res


def kernel(x, weight, bias):
    out, _ = run(x, weight, bias, trace=False)
    return out

